# revision 1
# baseline (speedup 1.0000x reference)
"""CapsuleLayer (dynamic routing) Trainium2 Bass kernel.

Reference computation (B=64, N=512, D=1024, NCAP=16, DCAP=64, R=3):
    u_hat = (x @ W).reshape(B, N, 16, 64).transpose(0, 2, 1, 3)
    b = 0
    for t in range(3):
        c = softmax(b, axis=capsule)                  # [B, 16, N]
        v = einsum('bij,bijk->bik', c, u_hat)         # [B, 16, 64]
        out = v / sqrt(sum(v^2, -1) + 1e-7)
        if t < 2: b = einsum('bik,bijk->bij', out, u_hat)

Key algebraic refactoring (never materialize the 68.7 GFLOP u_hat):
    v[b,i,:]  = (c[b,i,:] @ x[b]) @ W_blk[i]          # s = c@x  then  s@W blockdiag
    b[b,i,j]  = x[b,j,:] . (W_blk[i] @ out[b,i,:])    # wo = W_blk@out then x@wo
This cuts PE work ~14x. Sharding: data-parallel over batch, 8 batches/core,
W replicated. All big operands stored bf16 in SBUF, fp32 PSUM accumulation.
Col-tiling (tile_position) packs 4 batches' [K=128 x 32]-strip matmuls into
the PE array concurrently for the c@x and x@wo streams.

Layout conventions per core (BL = 8 local batches):
    x_sb  [128p, lb, jt, d]    p+128*jt = j           (bf16)
    xT_sb [128p, lb, dc, jf]   p+128*dc = d           (bf16, PE-transposed)
    W_sb  [128p, o, n]         W[o*128+p, n]          (bf16)
    WT_sb [128p, o, d]         W[d, o*128+p]          (bf16, host-transposed)
    S/v/output rows ordered (lb, i): row = lb*16 + i
    O_expT / wo-psum columns ordered (i, lb): col = i*8 + lb
The off-diagonal blocks of the dense S@W product are zeroed with a
host-provided block-diagonal mask; the masked [128, 1024] tile serves as
both the squash input and the zero-padded expanded-outputs operand of the
next routing iteration. The host extracts the diagonal blocks at the end.
"""

import numpy as np
import ml_dtypes

import concourse.bass as bass
import concourse.bacc as bacc
import concourse.mybir as mybir
import concourse.tile as tile
from concourse.bass_utils import run_bass_kernel_spmd
from concourse.masks import make_identity

NCORES = 8
B, N, D = 64, 512, 1024
NCAP, DCAP, R = 16, 64, 3
EPS = 1e-7
BL = B // NCORES          # local batches per core = 8
JT = N // 128             # j tiles = 4
DC = D // 128             # d chunks = 8
KC = D // 128             # k' chunks (NCAP*DCAP = 1024) = 8
QUADS = BL // 4           # col-tiling groups of 4 batches = 2

F32 = mybir.dt.float32
BF16 = mybir.dt.bfloat16
AX = mybir.AxisListType
ALU = mybir.AluOpType
ACT = mybir.ActivationFunctionType


def build_kernel(debug=False, reps=1, skip_b=False, skip_s=False, skip_out=False, dense_wo=True):
    nc = bacc.Bacc(
        "TRN2",
        target_bir_lowering=False,
        debug=False,
        enable_asserts=False,
        num_devices=NCORES,
    )

    x_dram = nc.dram_tensor("x", (BL, N, D), BF16, kind="ExternalInput")
    w_dram = nc.dram_tensor("W", (D, NCAP * DCAP), BF16, kind="ExternalInput")
    wt_dram = nc.dram_tensor("WT", (NCAP * DCAP, D), BF16, kind="ExternalInput")
    mask_dram = nc.dram_tensor("dmask", (128, NCAP * DCAP), F32, kind="ExternalInput")
    out_dram = nc.dram_tensor("out", (128, NCAP * DCAP), F32, kind="ExternalOutput")

    with tile.TileContext(nc) as tc:
        with (
            tc.tile_pool(name="const", bufs=1) as cpool,
            tc.tile_pool(name="work", bufs=1) as wpool,
            tc.tile_pool(name="work2", bufs=2) as w2pool,
            tc.tile_pool(name="ptp", bufs=2, space="PSUM") as tp_pool,
            tc.tile_pool(name="ps4", bufs=2, space="PSUM") as s4_pool,
        ):
            # ---------------- persistent SBUF tensors ----------------
            ident_bf = cpool.tile([128, 128], BF16, tag="identb")
            make_identity(nc, ident_bf)
            ident_f32 = cpool.tile([128, 128], F32, tag="identf")
            make_identity(nc, ident_f32)

            w_sb = cpool.tile([128, DC, NCAP * DCAP], BF16, tag="w")
            nc.sync.dma_start(w_sb[:], w_dram[:].rearrange("(o p) n -> p o n", p=128))
            wt_sb = cpool.tile([128, KC, D], BF16, tag="wt")
            nc.sync.dma_start(wt_sb[:], wt_dram[:].rearrange("(o p) d -> p o d", p=128))

            x_sb = cpool.tile([128, BL, JT, D], BF16, tag="x")
            for lb in range(BL):
                nc.sync.dma_start(
                    x_sb[:, lb],
                    x_dram[lb].rearrange("(jt p) d -> p jt d", p=128),
                )

            # Block-diagonal mask (see module docstring).
            dmask = cpool.tile([128, NCAP * DCAP], F32, tag="dmask")
            nc.sync.dma_start(dmask[:], mask_dram[:])

            eps_sb = cpool.tile([128, 1], F32, tag="eps")
            nc.gpsimd.memset(eps_sb[:], EPS)

            # c (routing weights) padded to 32 cols per (lb, jt) so col-tiled
            # matmul strips write full 32-partition rows (zeros in the pad).
            c_all = cpool.tile([128, BL, JT, 32], BF16, tag="c")
            nc.gpsimd.memset(c_all[:], 0.0)
            # woT[d, (lb, i)] padded the same way: [.., lb, 0:16] real.
            wot = cpool.tile([128, DC, BL, 32], BF16, tag="wot")
            nc.gpsimd.memset(wot[:], 0.0)

            # ---------------- x transposes (xT[d, j] per batch) ----------------
            xt_sb = cpool.tile([128, BL, DC, N], BF16, tag="xt")
            for lb in range(BL):
                for dc in range(DC):
                    pt = tp_pool.tile([128, 512], BF16, tag="tpb")
                    for jt in range(JT):
                        nc.tensor.transpose(
                            pt[:, jt * 128:(jt + 1) * 128],
                            x_sb[:, lb, jt, dc * 128:(dc + 1) * 128],
                            ident_bf,
                        )
                    if dc % 2 == 0:
                        nc.vector.tensor_copy(xt_sb[:, lb, dc], pt[:])
                    else:
                        nc.scalar.copy(xt_sb[:, lb, dc], pt[:])

            o_final = None
            for _rep in range(reps):
              for t in range(R):
                # ---------------- routing weights c ----------------
                if t == 0:
                    # b == 0  =>  uniform c. The 1/16 scale is irrelevant:
                    # squash normalizes it away (||v||^2 >> eps).
                    nc.gpsimd.memset(c_all[:, :, :, 0:NCAP], 1.0)
                elif skip_b:
                    nc.gpsimd.memset(c_all[:, :, :, 0:NCAP], 1.0)
                else:
                    # --- O_expT[k', (i,lb)] via PE transpose of masked outs.
                    # 4 chunks share a PSUM tile; the single copy permutes
                    # (lb,i) -> (i,lb) so each capsule pair is a contiguous
                    # 16-col block for the wo matmul. ---
                    oexpt = w2pool.tile([128, KC, 128], BF16, tag="oexpt")
                    for kh in range(2):
                        pt = tp_pool.tile([128, 4, 128], F32, tag="tpf")
                        for kq in range(4):
                            kc = kh * 4 + kq
                            nc.tensor.transpose(
                                pt[:, kq], o_final[:, kc * 128:(kc + 1) * 128],
                                ident_f32,
                            )
                        nc.any.tensor_copy(
                            out=oexpt[:, kh * 4:(kh + 1) * 4].rearrange(
                                "p kc (i l) -> p kc i l", l=BL),
                            in_=pt[:].rearrange("p kc (l i) -> p kc i l", l=BL),
                        )
                    # --- woT[d, (lb,i)] = sum_k' WT[k',d] * O_expT[k',(i,lb)].
                    if dense_wo:
                        # Dense: wo[(i,lb), d] = oexpt^T @ WT; the zero blocks
                        # of O_expT kill cross-capsule terms, so summing all
                        # k'-chunks is exact. 16 big MMs instead of 64 tiny
                        # LDW-bound ones; woT obtained by PE transpose.
                        pwf = s4_pool.tile([128, D], F32, tag="s4")
                        for kc in range(KC):
                            for nh in range(2):
                                nc.tensor.matmul(
                                    pwf[:, nh * 512:(nh + 1) * 512],
                                    oexpt[:, kc, :],
                                    wt_sb[:, kc, nh * 512:(nh + 1) * 512],
                                    start=(kc == 0), stop=(kc == KC - 1),
                                )
                        wo_sb = w2pool.tile([128, D], BF16, tag="wosb")
                        nc.vector.tensor_copy(wo_sb[:], pwf[:])
                        for dh in range(2):
                            pw = tp_pool.tile([128, 4, 128], BF16, tag="tpb")
                            for dq in range(4):
                                dc = dh * 4 + dq
                                nc.tensor.transpose(
                                    pw[:, dq], wo_sb[:, dc * 128:(dc + 1) * 128],
                                    ident_bf,
                                )
                            # pw cols are (i,lb); wot wants (lb,i)
                            nc.any.tensor_copy(
                                out=wot[:, dh * 4:(dh + 1) * 4, :, 0:NCAP],
                                in_=pw[:].rearrange("p dc (i l) -> p dc l i", l=BL),
                            )
                    else:
                      # Column block kc*16..kc*16+16 == capsules {2kc, 2kc+1}
                      # is fed only by k'-chunk kc (block-diagonal W). 4 d-chunks
                      # share a PSUM tile -> 2 copies per iteration. ---
                      for dh in range(2):
                        pw = tp_pool.tile([128, 4, 128], F32, tag="tpf")
                        for dq in range(4):
                            dc = dh * 4 + dq
                            for kc in range(KC):
                                nc.tensor.matmul(
                                    pw[:, dq, kc * 16:(kc + 1) * 16],
                                    wt_sb[:, kc, dc * 128:(dc + 1) * 128],
                                    oexpt[:, kc, kc * 16:(kc + 1) * 16],
                                    start=True, stop=True,
                                )
                        # pw cols are (i,lb); wot wants (lb,i)
                        nc.any.tensor_copy(
                            out=wot[:, dh * 4:(dh + 1) * 4, :, 0:NCAP],
                            in_=pw[:].rearrange("p dc (i l) -> p dc l i", l=BL),
                        )
                    # --- bT[i, j] per batch = sum_d woT[d,i] xT[d,j],
                    # col-tiled 4 batches per PSUM tile ---
                    b_all = w2pool.tile([128, BL, JT, NCAP], F32, tag="b")
                    for q in range(QUADS):
                        pb = tp_pool.tile([128, 512], F32, tag="tpf")
                        for dc in range(DC):
                            for lq in range(4):
                                lb = q * 4 + lq
                                nc.tensor.matmul(
                                    pb[32 * lq:32 * lq + 32, :],
                                    wot[:, dc, lb, :],
                                    xt_sb[:, lb, dc],
                                    start=(dc == 0), stop=(dc == DC - 1),
                                    tile_position=(0, 32 * lq),
                                    skip_group_check=True,
                                )
                        bt4 = wpool.tile([128, 512], F32, tag="bt4")
                        nc.vector.tensor_copy(bt4[:], pb[:])
                        # transpose to b[j, i] layout and scatter into b_all
                        ptb = tp_pool.tile([128, 4, 128], F32, tag="tpf")
                        for jt in range(JT):
                            nc.tensor.transpose(
                                ptb[:, jt],
                                bt4[:, jt * 128:(jt + 1) * 128],
                                ident_f32,
                            )
                        nc.any.tensor_copy(
                            out=b_all[:, q * 4:(q + 1) * 4, :, :].rearrange(
                                "p lq jt i -> p jt lq i"),
                            in_=ptb[:].rearrange(
                                "p jt (lq r) -> p jt lq r", lq=4)[..., 0:NCAP],
                        )
                    # --- softmax over capsule axis (free dim, batched).
                    # b stays within [-8, 8] for this model, so skipping the
                    # max-subtraction is safe in fp32. ---
                    bv = b_all[:].rearrange("p lb jt i -> p (lb jt) i")
                    nc.scalar.activation(bv, bv, ACT.Exp)
                    sumexp = wpool.tile([128, BL * JT], F32, tag="sumexp")
                    nc.vector.reduce_sum(sumexp[:], bv, axis=AX.X)
                    rec = wpool.tile([128, BL * JT], F32, tag="rec")
                    nc.vector.reciprocal(rec[:], sumexp[:])
                    nc.vector.tensor_tensor(
                        c_all[:, :, :, 0:NCAP].rearrange("p lb jt i -> p (lb jt) i"),
                        bv,
                        rec[:, :, None].to_broadcast((128, BL * JT, NCAP)),
                        ALU.mult,
                    )

                # ---------------- s = c @ x  (col-tiled, 4 batches/psum) -----
                st_all = w2pool.tile([128, DC, 128], BF16, tag="st")
                if skip_s:
                    nc.gpsimd.memset(st_all[:], 0.01)
                for q in (range(0) if skip_s else range(QUADS)):
                    ps = s4_pool.tile([128, D], F32, tag="s4")
                    for jt in range(JT):
                        for nh in range(2):
                            for lq in range(4):
                                lb = q * 4 + lq
                                nc.tensor.matmul(
                                    ps[32 * lq:32 * lq + 32,
                                       nh * 512:(nh + 1) * 512],
                                    c_all[:, lb, jt, :],
                                    x_sb[:, lb, jt, nh * 512:(nh + 1) * 512],
                                    start=(jt == 0), stop=(jt == JT - 1),
                                    tile_position=(0, 32 * lq),
                                    skip_group_check=True,
                                )
                    s4 = w2pool.tile([128, D], BF16, tag="s4sb")
                    nc.vector.tensor_copy(s4[:], ps[:])
                    # transpose into ST[d, (lb,i)]; 4 chunks per PSUM tile
                    for dh in range(2):
                        pst = tp_pool.tile([128, 4, 128], BF16, tag="tpb")
                        for dq in range(4):
                            dc = dh * 4 + dq
                            nc.tensor.transpose(
                                pst[:, dq], s4[:, dc * 128:(dc + 1) * 128],
                                ident_bf,
                            )
                        nc.any.tensor_copy(
                            out=st_all[:, dh * 4:(dh + 1) * 4,
                                       q * 64:(q + 1) * 64].rearrange(
                                "p dc (lq i) -> p dc lq i", lq=4),
                            in_=pst[:].rearrange(
                                "p dc (lq r) -> p dc lq r", lq=4)[..., 0:NCAP],
                        )

                # ------------- v' = S @ W (dense, diag-blocks used) ----------
                if skip_out:
                    o_tmp = w2pool.tile([128, NCAP * DCAP], F32, tag="ofull")
                    nc.gpsimd.memset(o_tmp[:], 0.02)
                    o_final = o_tmp
                    continue
                po = s4_pool.tile([128, NCAP * DCAP], F32, tag="s4")
                for dc in range(DC):
                    for nh in range(2):
                        nc.tensor.matmul(
                            po[:, nh * 512:(nh + 1) * 512],
                            st_all[:, dc],
                            w_sb[:, dc, nh * 512:(nh + 1) * 512],
                            start=(dc == 0), stop=(dc == DC - 1),
                        )
                # ------------- mask off-diag + squash ------------------------
                o_full = w2pool.tile([128, NCAP * DCAP], F32, tag="ofull")
                nc.vector.tensor_tensor(o_full[:], po[:], dmask[:], ALU.mult)
                sq_tmp = wpool.tile([128, NCAP * DCAP], F32, tag="sqtmp")
                ss = wpool.tile([128, 1], F32, tag="ss")
                nc.scalar.activation(sq_tmp[:], o_full[:], ACT.Square,
                                     accum_out=ss[:])
                sqv = wpool.tile([128, 1], F32, tag="sqv")
                nc.scalar.activation(sqv[:], ss[:], ACT.Sqrt, bias=eps_sb[:])
                rinv = wpool.tile([128, 1], F32, tag="rinv")
                nc.vector.reciprocal(rinv[:], sqv[:])
                nc.vector.tensor_scalar_mul(o_full[:], o_full[:], rinv[:])
                o_final = o_full

            # ---------------- write result ----------------
            nc.sync.dma_start(out_dram[:], o_final[:])

    nc.compile()
    return nc


_NC_CACHE = {}


def _get_nc(debug=False):
    key = bool(debug)
    if key not in _NC_CACHE:
        _NC_CACHE[key] = build_kernel(debug=key)
    return _NC_CACHE[key]


def block_diag_mask():
    """dmask[lb*NCAP+i, n] = 1.0 iff n // DCAP == i (capsule i's block)."""
    m = np.zeros((128, NCAP * DCAP), dtype=np.float32)
    for lb in range(BL):
        for i in range(NCAP):
            m[lb * NCAP + i, i * DCAP:(i + 1) * DCAP] = 1.0
    return m


def make_in_maps(x, W):
    """Host-side prep: shard x over batch, cast bf16, replicate W and W^T."""
    assert x.shape == (B, N, D) and W.shape[-2:] == (D, NCAP * DCAP)
    w2 = np.ascontiguousarray(W.reshape(D, NCAP * DCAP)).astype(ml_dtypes.bfloat16)
    wt = np.ascontiguousarray(W.reshape(D, NCAP * DCAP).T).astype(ml_dtypes.bfloat16)
    dm = block_diag_mask()
    xb = x.astype(ml_dtypes.bfloat16)
    in_maps = []
    for c in range(NCORES):
        in_maps.append({
            "x": np.ascontiguousarray(xb[c * BL:(c + 1) * BL]),
            "W": w2,
            "WT": wt,
            "dmask": dm,
        })
    return in_maps


def extract_out(core_out):
    """[128, 1024] masked tile -> [BL, NCAP, DCAP] (row lb*NCAP+i, block i)."""
    r = np.empty((BL, NCAP, DCAP), dtype=np.float32)
    for i in range(NCAP):
        r[:, i, :] = core_out[i::NCAP, i * DCAP:(i + 1) * DCAP]
    return r


def kernel(x, W):
    nc = _get_nc(debug=False)
    in_maps = make_in_maps(np.asarray(x), np.asarray(W))
    res = run_bass_kernel_spmd(nc, in_maps, list(range(NCORES)))
    out = np.concatenate([extract_out(r["out"]) for r in res.results], axis=0)
    return out.astype(np.float32)

